# revision 1
# baseline (speedup 1.0000x reference)
"""Trainium2 Bass kernel for nn_CPCircuitLayer.

Math: with all_indices the full cartesian grid (s = n // H, h = n % H),
    out[b, s, h] = sum_r seq_emb[b,s,r] * hid_emb[b,h,r] * cp[r]
                 = (seq_emb[b] @ diag(cp) @ hid_emb[b].T)[s, h]
where seq_emb[b] = X_b @ seq_W.T  (X_b = hidden_states[b], contract H)
      hid_emb[b] = X_b.T @ hid_W.T                        (contract S)

Sharding: 8 cores = (batch b, seq half) pairs. Each core receives X_b
fully (the hid factor contracts over all of S) with rows rotated so its
own seq half comes first, plus a host-transposed copy of that half
(xt = X_b[half].T), and computes
    hid_embT = (hid_W*cp) @ X_b          [R, H]
    seq_embT = seq_W @ X_b[half].T       [R, S/2]
    out_half = seq_embT.T @ hid_embT     [S/2, H]
writing its [512, 1024] slice of the output.

The device program is raw Bass (no Tile framework) with manual
semaphores: x tiles stream on the Sync HWDGE queue and xt tiles on the
Activation HWDGE queue in parallel, the PE consumes tiles as they
arrive (hid + seq interleaved), PSUM->SBUF copies alternate between the
Vector and Scalar engines, and the 8 output chunks DMA out through a
5-deep PSUM bank rotation. Matmuls run in FP32R (fp32 rounded to 12
mantissa bits, streamed at full PE rate); inputs are pre-rounded to the
FP32R bit format on the host so the device does no conversion work. A
few dummy matmuls at kernel start warm the PE HAM clock gate.
"""

import numpy as np

B, S, H, R = 4, 1024, 1024, 32
N_CORES = 8
SH = S // 2   # seq rows per core
KT = S // 128  # k-tiles over the contraction dims
MT = SH // 128  # row tiles in this core's seq half

_compiled = {}


def _np_fallback(hidden_states, all_indices, seq_W, hid_W, cp_weight):
    seq_emb = np.einsum("bsh,rh->bsr", hidden_states, seq_W)
    hid_emb = np.einsum("bsh,rs->bhr", hidden_states, hid_W)
    s_idx = all_indices[:, 0].astype(np.int64)
    h_idx = all_indices[:, 1].astype(np.int64)
    g_seq = seq_emb[:, s_idx, :]
    g_hid = hid_emb[:, h_idx, :]
    out = np.einsum("bnr,bnr,r->bn", g_seq, g_hid, cp_weight[0])
    return out.reshape(B, S, H).astype(np.float32)


def _round_f32r(a):
    """Round fp32 to the FP32R format (RNE at 12 mantissa bits), bit-exact
    with the device's own fp32->fp32r conversion."""
    b = np.ascontiguousarray(a, dtype=np.float32).view(np.uint32)
    r = (b + np.uint32(0x7FF) + ((b >> np.uint32(12)) & np.uint32(1))) \
        & np.uint32(0xFFFFF000)
    return r.view(np.float32)


def _wtile(w):
    """[K, R] -> [128, KT*R] tile layout, partition-contiguous."""
    return np.ascontiguousarray(
        w.reshape(KT, 128, R).transpose(1, 0, 2).reshape(128, KT * R))


def build_raw_program():
    import contextlib

    import concourse.bass as bass
    import concourse.mybir as mybir

    f32 = mybir.dt.float32
    f32r = mybir.dt.float32r

    nc = bass.Bass("TRN2", target_bir_lowering=False, debug=False,
                   num_devices=N_CORES, enable_partition_id=False)

    x_d = nc.dram_tensor("x", [S, H], f32r, kind="ExternalInput")
    xt_d = nc.dram_tensor("xt", [H, SH], f32r, kind="ExternalInput")
    w_d = nc.dram_tensor("w", [128, 2 * KT * R], f32r, kind="ExternalInput")
    out_d = nc.dram_tensor("out", [SH, H], f32, kind="ExternalOutput")

    with contextlib.ExitStack() as _xs:
        E = _xs.enter_context
        w_t = E(nc.sbuf_tensor([128, 2 * KT * R], f32r))  # [p, sw | hw]
        x_t = E(nc.sbuf_tensor([128, KT, H], f32r))
        xt_t = E(nc.sbuf_tensor([128, KT, SH], f32r))
        hid_sb = E(nc.sbuf_tensor([R, H], f32r))
        seq_sb = E(nc.sbuf_tensor([R, SH], f32r))
        o_sb = E(nc.sbuf_tensor([128, MT, H], f32))
        hid_ps = E(nc.psum_tensor([R, H], f32))        # 2 banks
        seq_ps = E(nc.psum_tensor([R, SH], f32))       # 1 bank
        o_ps = [E(nc.psum_tensor(f"o_ps{i}", [128, 512], f32))
                for i in range(5)]                     # 5 banks
        dma_sem = E(nc.semaphore("dma_sem"))
        w_sem = E(nc.semaphore("w_sem"))
        pe_sem = E(nc.semaphore("pe_sem"))
        dve_sem = E(nc.semaphore("dve_sem"))
        act_sem = E(nc.semaphore("act_sem"))
        x_sem = [E(nc.semaphore(f"x_sem{j}")) for j in range(KT)]
        xt_sem = [E(nc.semaphore(f"xt_sem{j}")) for j in range(KT)]
        block = E(nc.Block(no_gpsimd_drain=True))

        sw = lambda k: w_t.ap()[:, k * R:(k + 1) * R]
        hw = lambda k: w_t.ap()[:, KT * R + k * R:KT * R + (k + 1) * R]

        # dve ops: 1 = hid_c0, 2 = seq_c, 3.. = even out copies
        # act ops: 1 = hid_c1, 2.. = odd out copies

        @block.sync
        def _(sync):
            sync.dma_start(out=w_t.ap(), in_=w_d[:]).then_inc(w_sem, 16)
            for k in range(KT):
                sync.dma_start(
                    out=x_t.ap()[:, k, :],
                    in_=x_d[k * 128:(k + 1) * 128, :],
                ).then_inc(x_sem[k], 16)
            # even out chunks: dispatch once the DVE copy lands in o_sb
            for j in range(0, 2 * MT, 2):
                m, n = divmod(j, 2)
                sync.wait_ge(dve_sem, 3 + j // 2)
                sync.dma_start(
                    out=out_d[m * 128:(m + 1) * 128, n * 512:(n + 1) * 512],
                    in_=o_sb.ap()[:, m, n * 512:(n + 1) * 512],
                ).then_inc(dma_sem, 16)
            sync.wait_ge(dma_sem, 16 * 2 * MT)

        @block.tensor
        def _(tensor):
            # pe counts: per k: hid n0 = 3k+1, n1 = 3k+2, seq = 3k+3
            # hid_n0 done @22, n1 @23, seq @24; final: 25..32
            tensor.wait_ge(w_sem, 16)
            for _ in range(6):
                nc.tensor.matmul(o_ps[0].ap()[0:R, :], w_t.ap()[:, 0:R],
                                 w_t.ap()[:, 0:512], start=True, stop=True)

            for k in range(KT):
                tensor.wait_ge(x_sem[k], 16)
                for n in range(2):
                    nc.tensor.matmul(
                        hid_ps.ap()[:, n * 512:(n + 1) * 512],
                        hw(k), x_t.ap()[:, k, n * 512:(n + 1) * 512],
                        start=(k == 0), stop=(k == KT - 1),
                    ).then_inc(pe_sem, 1)
                tensor.wait_ge(xt_sem[k], 16)
                nc.tensor.matmul(
                    seq_ps.ap(), sw(k), xt_t.ap()[:, k, :],
                    start=(k == 0), stop=(k == KT - 1),
                ).then_inc(pe_sem, 1)
                if k >= 4 and k < KT - 1:
                    # keep the PE HAM activity window busy into the final burst
                    nc.tensor.matmul(o_ps[4].ap()[0:R, :], w_t.ap()[:, 0:R],
                                     w_t.ap()[:, 0:512], start=True, stop=True)

            tensor.wait_ge(dve_sem, 2)   # hid_c0 + seq_c
            for j in range(2 * MT):
                m, n = divmod(j, 2)
                if j == 1:
                    tensor.wait_ge(act_sem, 1)   # hid_c1 (odd j only)
                if j >= 5:
                    # WAR on recycled PSUM bank (5-deep rotation)
                    prev = j - 5
                    if prev % 2 == 0:
                        tensor.wait_ge(dve_sem, 3 + prev // 2)
                    else:
                        tensor.wait_ge(act_sem, 2 + (prev - 1) // 2)
                nc.tensor.matmul(
                    o_ps[j % 5].ap(),
                    seq_sb.ap()[:, m * 128:(m + 1) * 128],
                    hid_sb.ap()[:, n * 512:(n + 1) * 512],
                    start=True, stop=True,
                ).then_inc(pe_sem, 1)

        @block.vector
        def _(vector):
            vector.wait_ge(pe_sem, 22)
            nc.vector.tensor_copy(
                hid_sb.ap()[:, 0:512],
                hid_ps.ap()[:, 0:512].bitcast(f32)).then_inc(dve_sem, 1)
            vector.wait_ge(pe_sem, 24)
            nc.vector.tensor_copy(
                seq_sb.ap(), seq_ps.ap().bitcast(f32)).then_inc(dve_sem, 1)
            for j in range(0, 2 * MT, 2):   # even out copies
                m, n = divmod(j, 2)
                vector.wait_ge(pe_sem, 24 + j + 1)
                nc.vector.tensor_copy(
                    o_sb.ap()[:, m, n * 512:(n + 1) * 512],
                    o_ps[j % 5].ap(),
                ).then_inc(dve_sem, 1)

        @block.scalar
        def _(scalar):
            # xt loads dispatch from the second HWDGE queue (Activation)
            for k in range(KT):
                scalar.dma_start(
                    out=xt_t.ap()[:, k, :],
                    in_=xt_d[k * 128:(k + 1) * 128, :],
                ).then_inc(xt_sem[k], 16)
            # dummy copy to pull the lazy ACT table load off the critical path
            scalar.wait_ge(w_sem, 16)
            nc.scalar.copy(o_sb.ap()[:, 0, 0:R], w_t.ap()[:, 0:R])
            scalar.wait_ge(pe_sem, 23)
            nc.scalar.copy(
                hid_sb.ap()[:, 512:1024],
                hid_ps.ap()[:, 512:1024].bitcast(f32)).then_inc(act_sem, 1)
            for j in range(1, 2 * MT, 2):   # odd out copies + own-queue DMA
                m, n = divmod(j, 2)
                scalar.wait_ge(pe_sem, 24 + j + 1)
                nc.scalar.copy(
                    o_sb.ap()[:, m, n * 512:(n + 1) * 512],
                    o_ps[j % 5].ap(),
                ).then_inc(act_sem, 1)
                scalar.dma_start(
                    out=out_d[m * 128:(m + 1) * 128, n * 512:(n + 1) * 512],
                    in_=o_sb.ap()[:, m, n * 512:(n + 1) * 512],
                ).then_inc(dma_sem, 16)

    return nc


def _get_program():
    if "nc" not in _compiled:
        _compiled["nc"] = build_raw_program()
    return _compiled["nc"]


def _make_in_maps(hidden_states, seq_W, hid_W, cp_weight):
    swT = _wtile(np.ascontiguousarray(seq_W.T))                    # [128, 256]
    hwT_rows = np.ascontiguousarray((hid_W * cp_weight[0][:, None]).T)  # [S, R]
    # per-half row rotation: own seq half first (hid contraction over S is
    # order-invariant as long as x rows and hw rows permute together)
    w_rot = [
        _round_f32r(np.concatenate([swT, _wtile(np.concatenate(
            [hwT_rows[half * SH:], hwT_rows[:half * SH]], axis=0))], axis=1))
        for half in range(2)
    ]
    in_maps = []
    for c in range(N_CORES):
        b, half = divmod(c, 2)
        xb = _round_f32r(hidden_states[b])
        if half:
            xb = np.ascontiguousarray(
                np.concatenate([xb[SH:], xb[:SH]], axis=0))
        in_maps.append({
            "x": xb,
            "xt": np.ascontiguousarray(xb[:SH, :].T),
            "w": w_rot[half],
        })
    return in_maps


def kernel(hidden_states, all_indices, seq_W, hid_W, cp_weight):
    hidden_states = np.asarray(hidden_states, dtype=np.float32)
    seq_W = np.asarray(seq_W, dtype=np.float32)
    hid_W = np.asarray(hid_W, dtype=np.float32)
    cp_weight = np.asarray(cp_weight, dtype=np.float32)
    idx = np.asarray(all_indices)

    # The reference's all_indices is always the full cartesian grid; verify
    # cheaply and fall back to a host path if ever not.
    n = np.arange(S * H, dtype=idx.dtype)
    if idx.shape != (S * H, 2) or not (
        np.array_equal(idx[:, 0], n // H) and np.array_equal(idx[:, 1], n % H)
    ):
        return _np_fallback(hidden_states, idx, seq_W, hid_W, cp_weight)

    from concourse.bass_utils import run_bass_kernel_spmd

    nc = _get_program()
    in_maps = _make_in_maps(hidden_states, seq_W, hid_W, cp_weight)
    res = run_bass_kernel_spmd(nc, in_maps, list(range(N_CORES)))

    out = np.empty((B, S, H), dtype=np.float32)
    for c in range(N_CORES):
        b, half = divmod(c, 2)
        out[b, half * SH:(half + 1) * SH, :] = res.results[c]["out"]
    return out



# revision 7
# speedup vs baseline: 1.1482x; 1.1482x over previous
"""Trainium2 Bass kernel for nn_CPCircuitLayer.

Math: with all_indices the full cartesian grid (s = n // H, h = n % H),
    out[b, s, h] = sum_r seq_emb[b,s,r] * hid_emb[b,h,r] * cp[r]
                 = (seq_emb[b] @ diag(cp) @ hid_emb[b].T)[s, h]
where seq_emb[b] = X_b @ seq_W.T  (X_b = hidden_states[b], contract H)
      hid_emb[b] = X_b.T @ hid_W.T                        (contract S)

Sharding: 8 cores = (batch b, seq half) pairs. Each core receives X_b
fully (the hid factor contracts over all of S) with rows rotated so its
own seq half comes first, plus a host-transposed copy of that half
(xt = X_b[half].T), and computes
    hid_embT = (hid_W*cp) @ X_b          [R, H]
    seq_embT = seq_W @ X_b[half].T       [R, S/2]
    out_half = seq_embT.T @ hid_embT     [S/2, H]
writing its [512, 1024] slice of the output.

All device data is FP16 (inputs rounded on host; PSUM accumulation is
FP32; output written FP16 and upcast on host) — rel err ~6e-4, and it
halves HBM traffic vs fp32 while tripling PE throughput vs the fp32
HIGH-precision matmul path. The device program is raw Bass with manual
semaphores: x streams in 2-k-tile chunks split across the Sync and
Scalar HWDGE queues (aggregate ~410 GB/s), the PE consumes chunks as
they arrive (hid + seq interleaved), PSUM->SBUF copies alternate
between the Vector and Scalar engines, and the 4 output row-blocks DMA
out through a 5-deep PSUM bank rotation. Dummy matmuls at kernel start
and mid-stream keep the PE HAM clock gate warm for the final burst.
"""

import numpy as np

B, S, H, R = 4, 1024, 1024, 32
N_CORES = 8
SH = S // 2   # seq rows per core
KT = S // 128  # k-tiles over the contraction dims
MT = SH // 128  # row tiles in this core's seq half

_compiled = {}


def _np_fallback(hidden_states, all_indices, seq_W, hid_W, cp_weight):
    seq_emb = np.einsum("bsh,rh->bsr", hidden_states, seq_W)
    hid_emb = np.einsum("bsh,rs->bhr", hidden_states, hid_W)
    s_idx = all_indices[:, 0].astype(np.int64)
    h_idx = all_indices[:, 1].astype(np.int64)
    g_seq = seq_emb[:, s_idx, :]
    g_hid = hid_emb[:, h_idx, :]
    out = np.einsum("bnr,bnr,r->bn", g_seq, g_hid, cp_weight[0])
    return out.reshape(B, S, H).astype(np.float32)


def _ptile(a, kt, w):
    """[kt*128, w] -> [128, kt, w] partition-major tile layout."""
    return np.ascontiguousarray(
        a.reshape(kt, 128, w).transpose(1, 0, 2))


def _wtile(w):
    """[K, R] -> [128, KT*R] tile layout, partition-contiguous."""
    return np.ascontiguousarray(
        w.reshape(KT, 128, R).transpose(1, 0, 2).reshape(128, KT * R))


def build_raw_program():
    import contextlib

    import concourse.bass as bass
    import concourse.mybir as mybir

    f16 = mybir.dt.float16
    f32 = mybir.dt.float32

    nc = bass.Bass("TRN2", target_bir_lowering=False, debug=False,
                   num_devices=N_CORES, enable_partition_id=False)

    x_d = nc.dram_tensor("x", [128, KT, H], f16, kind="ExternalInput")
    xt_d = nc.dram_tensor("xt", [128, KT, SH], f16, kind="ExternalInput")
    w_d = nc.dram_tensor("w", [128, 2 * KT * R], f16, kind="ExternalInput")
    out_d = nc.dram_tensor("out", [128, MT, H], f16, kind="ExternalOutput")

    with contextlib.ExitStack() as _xs:
        E = _xs.enter_context
        w_t = E(nc.sbuf_tensor([128, 2 * KT * R], f16))  # [p, sw | hw]
        x_t = E(nc.sbuf_tensor([128, KT, H], f16))
        xt_t = E(nc.sbuf_tensor([128, KT, SH], f16))
        hid_sb = E(nc.sbuf_tensor([R, H], f16))
        seq_sb = E(nc.sbuf_tensor([R, SH], f16))
        o_sb = E(nc.sbuf_tensor([128, MT, H], f16))
        scr_sb = E(nc.sbuf_tensor([128, R], f16))
        hid_ps = E(nc.psum_tensor([R, H], f32))        # 2 banks
        seq_ps = E(nc.psum_tensor([R, SH], f32))       # 1 bank
        o_ps = [E(nc.psum_tensor(f"o_ps{i}", [128, 512], f32))
                for i in range(5)]                     # 5 banks
        dma_sem = E(nc.semaphore("dma_sem"))
        w_sem = E(nc.semaphore("w_sem"))
        pe_sem = E(nc.semaphore("pe_sem"))
        dve_sem = E(nc.semaphore("dve_sem"))
        act_sem = E(nc.semaphore("act_sem"))
        xc_sem = [E(nc.semaphore(f"xc_sem{j}")) for j in range(4)]
        xtc_sem = [E(nc.semaphore(f"xtc_sem{j}")) for j in range(2)]
        block = E(nc.Block(no_gpsimd_drain=True))

        sw = lambda k: w_t.ap()[:, k * R:(k + 1) * R]
        hw = lambda k: w_t.ap()[:, KT * R + k * R:KT * R + (k + 1) * R]

        # x chunks: c0 = k0-1, c1 = k2-3, c2 = k4-5 (sync); c3 = k6-7 (scalar)
        # dve ops: 1 = hid_c0, 2 = seq_c, 3.. = (m,0) out copies
        # act ops: 1 = hid_c1, 2.. = (m,1) out copies

        @block.sync
        def _(sync):
            sync.dma_start(out=w_t.ap(), in_=w_d[:]).then_inc(w_sem, 16)
            for c in range(3):
                sync.dma_start(
                    out=x_t.ap()[:, 2 * c:2 * c + 2, :],
                    in_=x_d[:, 2 * c:2 * c + 2, :],
                ).then_inc(xc_sem[c], 16)
            # out rows 0, 2: dispatch once both halves copied to o_sb
            for m in (0, 2):
                sync.wait_ge(dve_sem, 3 + m)
                sync.wait_ge(act_sem, 2 + m)
                sync.dma_start(
                    out=out_d[:, m, :],
                    in_=o_sb.ap()[:, m, :],
                ).then_inc(dma_sem, 16)
            sync.wait_ge(dma_sem, 16 * MT)

        @block.tensor
        def _(tensor):
            # pe counts: per k: hid n0 = 3k+1, n1 = 3k+2, seq = 3k+3
            # hid_n0 done @22, n1 @23, seq @24; final: 25..32
            tensor.wait_ge(w_sem, 16)
            for _ in range(6):
                nc.tensor.matmul(o_ps[0].ap()[0:R, :], w_t.ap()[:, 0:R],
                                 w_t.ap()[:, 0:512], start=True, stop=True)

            for k in range(KT):
                if k < 6:
                    tensor.wait_ge(xc_sem[k // 2], 16)
                else:
                    tensor.wait_ge(xc_sem[3], 16)
                for n in range(2):
                    nc.tensor.matmul(
                        hid_ps.ap()[:, n * 512:(n + 1) * 512],
                        hw(k), x_t.ap()[:, k, n * 512:(n + 1) * 512],
                        start=(k == 0), stop=(k == KT - 1),
                    ).then_inc(pe_sem, 1)
                tensor.wait_ge(xtc_sem[k // 4], 16)
                nc.tensor.matmul(
                    seq_ps.ap(), sw(k), xt_t.ap()[:, k, :],
                    start=(k == 0), stop=(k == KT - 1),
                ).then_inc(pe_sem, 1)
                if k >= 4 and k < KT - 1:
                    # keep the PE HAM activity window busy into the final burst
                    nc.tensor.matmul(o_ps[4].ap()[0:R, :], w_t.ap()[:, 0:R],
                                     w_t.ap()[:, 0:512], start=True, stop=True)

            tensor.wait_ge(dve_sem, 2)   # hid_c0 + seq_c
            for j in range(2 * MT):
                m, n = divmod(j, 2)
                if j == 1:
                    tensor.wait_ge(act_sem, 1)   # hid_c1 (odd j only)
                if j >= 5:
                    # WAR on recycled PSUM bank (5-deep rotation)
                    prev = j - 5
                    if prev % 2 == 0:
                        tensor.wait_ge(dve_sem, 3 + prev // 2)
                    else:
                        tensor.wait_ge(act_sem, 2 + (prev - 1) // 2)
                nc.tensor.matmul(
                    o_ps[j % 5].ap(),
                    seq_sb.ap()[:, m * 128:(m + 1) * 128],
                    hid_sb.ap()[:, n * 512:(n + 1) * 512],
                    start=True, stop=True,
                ).then_inc(pe_sem, 1)

        @block.vector
        def _(vector):
            vector.wait_ge(pe_sem, 22)
            nc.vector.tensor_copy(
                hid_sb.ap()[:, 0:512],
                hid_ps.ap()[:, 0:512]).then_inc(dve_sem, 1)
            vector.wait_ge(pe_sem, 24)
            nc.vector.tensor_copy(
                seq_sb.ap(), seq_ps.ap()).then_inc(dve_sem, 1)
            for j in range(0, 2 * MT, 2):   # (m, 0) out copies
                m, n = divmod(j, 2)
                vector.wait_ge(pe_sem, 24 + j + 1)
                nc.vector.tensor_copy(
                    o_sb.ap()[:, m, n * 512:(n + 1) * 512],
                    o_ps[j % 5].ap(),
                ).then_inc(dve_sem, 1)

        @block.scalar
        def _(scalar):
            # second HWDGE queue: xt chunks, the last x chunk, odd out rows
            for c in range(2):
                scalar.dma_start(
                    out=xt_t.ap()[:, 4 * c:4 * c + 4, :],
                    in_=xt_d[:, 4 * c:4 * c + 4, :],
                ).then_inc(xtc_sem[c], 16)
            scalar.dma_start(
                out=x_t.ap()[:, 6:8, :],
                in_=x_d[:, 6:8, :],
            ).then_inc(xc_sem[3], 16)
            # dummy copy to pull the lazy ACT table load off the critical path
            scalar.wait_ge(w_sem, 16)
            nc.scalar.copy(scr_sb.ap(), w_t.ap()[:, 0:R])
            scalar.wait_ge(pe_sem, 23)
            nc.scalar.copy(
                hid_sb.ap()[:, 512:1024],
                hid_ps.ap()[:, 512:1024]).then_inc(act_sem, 1)
            for j in range(1, 2 * MT, 2):   # (m, 1) out copies; DMA odd rows
                m, n = divmod(j, 2)
                scalar.wait_ge(pe_sem, 24 + j + 1)
                nc.scalar.copy(
                    o_sb.ap()[:, m, n * 512:(n + 1) * 512],
                    o_ps[j % 5].ap(),
                ).then_inc(act_sem, 1)
                if m % 2 == 1:
                    scalar.wait_ge(dve_sem, 3 + m)   # (m, 0) half from DVE
                    scalar.wait_ge(act_sem, 2 + m)   # own copy (sim-visible)
                    scalar.dma_start(
                        out=out_d[:, m, :],
                        in_=o_sb.ap()[:, m, :],
                    ).then_inc(dma_sem, 16)

    return nc


def _get_program():
    if "nc" not in _compiled:
        _compiled["nc"] = build_raw_program()
    return _compiled["nc"]


def _make_in_maps(hidden_states, seq_W, hid_W, cp_weight):
    swT = _wtile(np.ascontiguousarray(seq_W.T))                    # [128, 256]
    hwT_rows = np.ascontiguousarray((hid_W * cp_weight[0][:, None]).T)  # [S, R]
    # per-half row rotation: own seq half first (hid contraction over S is
    # order-invariant as long as x rows and hw rows permute together)
    w_rot = [
        np.concatenate([swT, _wtile(np.concatenate(
            [hwT_rows[half * SH:], hwT_rows[:half * SH]], axis=0))],
            axis=1).astype(np.float16)
        for half in range(2)
    ]
    in_maps = []
    for c in range(N_CORES):
        b, half = divmod(c, 2)
        xb = hidden_states[b]
        if half:
            xb = np.concatenate([xb[SH:], xb[:SH]], axis=0)
        xb = xb.astype(np.float16)
        in_maps.append({
            "x": _ptile(xb, KT, H),
            "xt": _ptile(np.ascontiguousarray(xb[:SH, :].T), KT, SH),
            "w": w_rot[half],
        })
    return in_maps


def kernel(hidden_states, all_indices, seq_W, hid_W, cp_weight):
    hidden_states = np.asarray(hidden_states, dtype=np.float32)
    seq_W = np.asarray(seq_W, dtype=np.float32)
    hid_W = np.asarray(hid_W, dtype=np.float32)
    cp_weight = np.asarray(cp_weight, dtype=np.float32)
    idx = np.asarray(all_indices)

    # The reference's all_indices is always the full cartesian grid; verify
    # cheaply and fall back to a host path if ever not.
    n = np.arange(S * H, dtype=idx.dtype)
    if idx.shape != (S * H, 2) or not (
        np.array_equal(idx[:, 0], n // H) and np.array_equal(idx[:, 1], n % H)
    ):
        return _np_fallback(hidden_states, idx, seq_W, hid_W, cp_weight)

    from concourse.bass_utils import run_bass_kernel_spmd

    nc = _get_program()
    in_maps = _make_in_maps(hidden_states, seq_W, hid_W, cp_weight)
    res = run_bass_kernel_spmd(nc, in_maps, list(range(N_CORES)))

    out = np.empty((B, S, H), dtype=np.float32)
    for c in range(N_CORES):
        b, half = divmod(c, 2)
        o = res.results[c]["out"]  # [128, MT, H] f16
        out[b, half * SH:(half + 1) * SH, :] = (
            o.transpose(1, 0, 2).reshape(SH, H).astype(np.float32))
    return out


# revision 8
# speedup vs baseline: 1.1740x; 1.0224x over previous
"""Trainium2 Bass kernel for nn_CPCircuitLayer.

Math: with all_indices the full cartesian grid (s = n // H, h = n % H),
    out[b, s, h] = sum_r seq_emb[b,s,r] * hid_emb[b,h,r] * cp[r]
                 = (seq_emb[b] @ diag(cp) @ hid_emb[b].T)[s, h]
where seq_emb[b] = X_b @ seq_W.T  (X_b = hidden_states[b], contract H)
      hid_emb[b] = X_b.T @ hid_W.T                        (contract S)

Sharding: 8 cores = (batch b, seq half) pairs. Each core receives X_b
fully (the hid factor contracts over all of S) with rows rotated so its
own seq half comes first, plus a host-transposed copy of that half
(xt = X_b[half].T), and computes
    hid_embT = (hid_W*cp) @ X_b          [R, H]
    seq_embT = seq_W @ X_b[half].T       [R, S/2]
    out_half = seq_embT.T @ hid_embT     [S/2, H]
writing its [512, 1024] slice of the output.

All device data is FP16 (inputs rounded on host; PSUM accumulation is
FP32; output written FP16 and upcast on host) — rel err ~6e-4, and it
halves HBM traffic vs fp32 while tripling PE throughput vs the fp32
HIGH-precision matmul path. Every DMA moves a region that is fully
contiguous in DRAM (the host pre-packs chunk-major layouts) — strided
DRAM access patterns measurably halve HBM DMA throughput. x streams in
2-k-tile 512KB chunks split across the Sync and Scalar HWDGE queues
(aggregate ~410 GB/s), the PE consumes chunks as they arrive (hid + seq
interleaved), PSUM->SBUF copies alternate between the Vector and Scalar
engines, and the 4 output row-blocks DMA out through a 5-deep PSUM bank
rotation. Dummy matmuls at kernel start and mid-stream keep the PE HAM
clock gate warm for the final burst.
"""

import numpy as np

B, S, H, R = 4, 1024, 1024, 32
N_CORES = 8
SH = S // 2   # seq rows per core
KT = S // 128  # k-tiles over the contraction dims
MT = SH // 128  # row tiles in this core's seq half

_compiled = {}


def _np_fallback(hidden_states, all_indices, seq_W, hid_W, cp_weight):
    seq_emb = np.einsum("bsh,rh->bsr", hidden_states, seq_W)
    hid_emb = np.einsum("bsh,rs->bhr", hidden_states, hid_W)
    s_idx = all_indices[:, 0].astype(np.int64)
    h_idx = all_indices[:, 1].astype(np.int64)
    g_seq = seq_emb[:, s_idx, :]
    g_hid = hid_emb[:, h_idx, :]
    out = np.einsum("bnr,bnr,r->bn", g_seq, g_hid, cp_weight[0])
    return out.reshape(B, S, H).astype(np.float32)


def _chunk_pack(a, tiles_per_chunk):
    """[T*128, w] -> [T//c * 128, c*w]: DMA chunks of c k-tiles, each chunk
    a fully contiguous [128, c*w] block (partition p holds tiles' row p)."""
    t = a.shape[0] // 128
    w = a.shape[1]
    c = tiles_per_chunk
    return np.ascontiguousarray(
        a.reshape(t // c, c, 128, w).transpose(0, 2, 1, 3)
        .reshape(t // c * 128, c * w))


def _wtile(w):
    """[K, R] -> [128, KT*R] tile layout, partition-contiguous."""
    return np.ascontiguousarray(
        w.reshape(KT, 128, R).transpose(1, 0, 2).reshape(128, KT * R))


def build_raw_program():
    import contextlib

    import concourse.bass as bass
    import concourse.mybir as mybir

    f16 = mybir.dt.float16
    f32 = mybir.dt.float32

    nc = bass.Bass("TRN2", target_bir_lowering=False, debug=False,
                   num_devices=N_CORES, enable_partition_id=False)

    # x: 4 chunks of 2 k-tiles, each chunk contiguous [128, 2048]
    x_d = nc.dram_tensor("x", [4 * 128, 2 * H], f16, kind="ExternalInput")
    # xt: 2 chunks of 4 k-tiles, each chunk contiguous [128, 2048]
    xt_d = nc.dram_tensor("xt", [2 * 128, 4 * SH], f16, kind="ExternalInput")
    w_d = nc.dram_tensor("w", [128, 2 * KT * R], f16, kind="ExternalInput")
    # out: this core's seq half, plain row-major [512, 1024]
    out_d = nc.dram_tensor("out", [SH, H], f16, kind="ExternalOutput")

    with contextlib.ExitStack() as _xs:
        E = _xs.enter_context
        w_t = E(nc.sbuf_tensor([128, 2 * KT * R], f16))  # [p, sw | hw]
        x_t = E(nc.sbuf_tensor([128, KT * H], f16))
        xt_t = E(nc.sbuf_tensor([128, KT * SH], f16))
        hid_sb = E(nc.sbuf_tensor([R, H], f16))
        seq_sb = E(nc.sbuf_tensor([R, SH], f16))
        o_sb = E(nc.sbuf_tensor([128, MT * H], f16))
        scr_sb = E(nc.sbuf_tensor([128, R], f16))
        hid_ps = E(nc.psum_tensor([R, H], f32))        # 2 banks
        seq_ps = E(nc.psum_tensor([R, SH], f32))       # 1 bank
        o_ps = [E(nc.psum_tensor(f"o_ps{i}", [128, 512], f32))
                for i in range(5)]                     # 5 banks
        dma_sem = E(nc.semaphore("dma_sem"))
        w_sem = E(nc.semaphore("w_sem"))
        pe_sem = E(nc.semaphore("pe_sem"))
        dve_sem = E(nc.semaphore("dve_sem"))
        act_sem = E(nc.semaphore("act_sem"))
        xc_sem = [E(nc.semaphore(f"xc_sem{j}")) for j in range(4)]
        xtc_sem = [E(nc.semaphore(f"xtc_sem{j}")) for j in range(2)]
        block = E(nc.Block(no_gpsimd_drain=True))

        sw = lambda k: w_t.ap()[:, k * R:(k + 1) * R]
        hw = lambda k: w_t.ap()[:, KT * R + k * R:KT * R + (k + 1) * R]
        xk = lambda k, n: x_t.ap()[:, k * H + n * 512:k * H + (n + 1) * 512]
        xtk = lambda k: xt_t.ap()[:, k * SH:(k + 1) * SH]
        ob = lambda m, n: o_sb.ap()[:, m * H + n * 512:m * H + (n + 1) * 512]

        # x chunks: c0 = k0-1, c1 = k2-3, c2 = k4-5 (sync); c3 = k6-7 (scalar)
        # dve ops: 1 = hid_c0, 2 = seq_c, 3.. = (m,0) out copies
        # act ops: 1 = hid_c1, 2.. = (m,1) out copies

        @block.sync
        def _(sync):
            sync.dma_start(out=w_t.ap(), in_=w_d[:]).then_inc(w_sem, 16)
            for c in range(3):
                sync.dma_start(
                    out=x_t.ap()[:, c * 2 * H:(c + 1) * 2 * H],
                    in_=x_d[c * 128:(c + 1) * 128, :],
                ).then_inc(xc_sem[c], 16)
            # out rows 0, 2: dispatch once both halves copied to o_sb
            for m in (0, 2):
                sync.wait_ge(dve_sem, 3 + m)
                sync.wait_ge(act_sem, 2 + m)
                sync.dma_start(
                    out=out_d[m * 128:(m + 1) * 128, :],
                    in_=o_sb.ap()[:, m * H:(m + 1) * H],
                ).then_inc(dma_sem, 16)
            sync.wait_ge(dma_sem, 16 * MT)

        @block.tensor
        def _(tensor):
            # pe counts: per k: hid n0 = 3k+1, n1 = 3k+2, seq = 3k+3
            # hid_n0 done @22, n1 @23, seq @24; final: 25..32
            tensor.wait_ge(w_sem, 16)
            for _ in range(6):
                nc.tensor.matmul(o_ps[0].ap()[0:R, :], w_t.ap()[:, 0:R],
                                 w_t.ap()[:, 0:512], start=True, stop=True)

            for k in range(KT):
                if k < 6:
                    tensor.wait_ge(xc_sem[k // 2], 16)
                else:
                    tensor.wait_ge(xc_sem[3], 16)
                for n in range(2):
                    nc.tensor.matmul(
                        hid_ps.ap()[:, n * 512:(n + 1) * 512],
                        hw(k), xk(k, n),
                        start=(k == 0), stop=(k == KT - 1),
                    ).then_inc(pe_sem, 1)
                tensor.wait_ge(xtc_sem[k // 4], 16)
                nc.tensor.matmul(
                    seq_ps.ap(), sw(k), xtk(k),
                    start=(k == 0), stop=(k == KT - 1),
                ).then_inc(pe_sem, 1)
                if k >= 4 and k < KT - 1:
                    # keep the PE HAM activity window busy into the final burst
                    nc.tensor.matmul(o_ps[4].ap()[0:R, :], w_t.ap()[:, 0:R],
                                     w_t.ap()[:, 0:512], start=True, stop=True)

            tensor.wait_ge(dve_sem, 2)   # hid_c0 + seq_c
            for j in range(2 * MT):
                m, n = divmod(j, 2)
                if j == 1:
                    tensor.wait_ge(act_sem, 1)   # hid_c1 (odd j only)
                if j >= 5:
                    # WAR on recycled PSUM bank (5-deep rotation)
                    prev = j - 5
                    if prev % 2 == 0:
                        tensor.wait_ge(dve_sem, 3 + prev // 2)
                    else:
                        tensor.wait_ge(act_sem, 2 + (prev - 1) // 2)
                nc.tensor.matmul(
                    o_ps[j % 5].ap(),
                    seq_sb.ap()[:, m * 128:(m + 1) * 128],
                    hid_sb.ap()[:, n * 512:(n + 1) * 512],
                    start=True, stop=True,
                ).then_inc(pe_sem, 1)

        @block.vector
        def _(vector):
            vector.wait_ge(pe_sem, 22)
            nc.vector.tensor_copy(
                hid_sb.ap()[:, 0:512],
                hid_ps.ap()[:, 0:512]).then_inc(dve_sem, 1)
            vector.wait_ge(pe_sem, 24)
            nc.vector.tensor_copy(
                seq_sb.ap(), seq_ps.ap()).then_inc(dve_sem, 1)
            for j in range(0, 2 * MT, 2):   # (m, 0) out copies
                m, n = divmod(j, 2)
                vector.wait_ge(pe_sem, 24 + j + 1)
                nc.vector.tensor_copy(
                    ob(m, n), o_ps[j % 5].ap(),
                ).then_inc(dve_sem, 1)

        @block.scalar
        def _(scalar):
            # second HWDGE queue: xt chunks, the last x chunk, odd out rows
            for c in range(2):
                scalar.dma_start(
                    out=xt_t.ap()[:, c * 4 * SH:(c + 1) * 4 * SH],
                    in_=xt_d[c * 128:(c + 1) * 128, :],
                ).then_inc(xtc_sem[c], 16)
            scalar.dma_start(
                out=x_t.ap()[:, 6 * H:8 * H],
                in_=x_d[3 * 128:4 * 128, :],
            ).then_inc(xc_sem[3], 16)
            # dummy copy to pull the lazy ACT table load off the critical path
            scalar.wait_ge(w_sem, 16)
            nc.scalar.copy(scr_sb.ap(), w_t.ap()[:, 0:R])
            scalar.wait_ge(pe_sem, 23)
            nc.scalar.copy(
                hid_sb.ap()[:, 512:1024],
                hid_ps.ap()[:, 512:1024]).then_inc(act_sem, 1)
            for j in range(1, 2 * MT, 2):   # (m, 1) out copies; DMA odd rows
                m, n = divmod(j, 2)
                scalar.wait_ge(pe_sem, 24 + j + 1)
                nc.scalar.copy(
                    ob(m, n), o_ps[j % 5].ap(),
                ).then_inc(act_sem, 1)
                if m % 2 == 1:
                    scalar.wait_ge(dve_sem, 3 + m)   # (m, 0) half from DVE
                    scalar.wait_ge(act_sem, 2 + m)   # own copy (sim-visible)
                    scalar.dma_start(
                        out=out_d[m * 128:(m + 1) * 128, :],
                        in_=o_sb.ap()[:, m * H:(m + 1) * H],
                    ).then_inc(dma_sem, 16)

    return nc


def _get_program():
    if "nc" not in _compiled:
        _compiled["nc"] = build_raw_program()
    return _compiled["nc"]


def _make_in_maps(hidden_states, seq_W, hid_W, cp_weight):
    swT = _wtile(np.ascontiguousarray(seq_W.T))                    # [128, 256]
    hwT_rows = np.ascontiguousarray((hid_W * cp_weight[0][:, None]).T)  # [S, R]
    # per-half row rotation: own seq half first (hid contraction over S is
    # order-invariant as long as x rows and hw rows permute together)
    w_rot = [
        np.concatenate([swT, _wtile(np.concatenate(
            [hwT_rows[half * SH:], hwT_rows[:half * SH]], axis=0))],
            axis=1).astype(np.float16)
        for half in range(2)
    ]
    in_maps = []
    for c in range(N_CORES):
        b, half = divmod(c, 2)
        xb = hidden_states[b]
        if half:
            xb = np.concatenate([xb[SH:], xb[:SH]], axis=0)
        xb = xb.astype(np.float16)
        in_maps.append({
            "x": _chunk_pack(xb, 2),
            "xt": _chunk_pack(np.ascontiguousarray(xb[:SH, :].T), 4),
            "w": w_rot[half],
        })
    return in_maps


def kernel(hidden_states, all_indices, seq_W, hid_W, cp_weight):
    hidden_states = np.asarray(hidden_states, dtype=np.float32)
    seq_W = np.asarray(seq_W, dtype=np.float32)
    hid_W = np.asarray(hid_W, dtype=np.float32)
    cp_weight = np.asarray(cp_weight, dtype=np.float32)
    idx = np.asarray(all_indices)

    # The reference's all_indices is always the full cartesian grid; verify
    # cheaply and fall back to a host path if ever not.
    n = np.arange(S * H, dtype=idx.dtype)
    if idx.shape != (S * H, 2) or not (
        np.array_equal(idx[:, 0], n // H) and np.array_equal(idx[:, 1], n % H)
    ):
        return _np_fallback(hidden_states, idx, seq_W, hid_W, cp_weight)

    from concourse.bass_utils import run_bass_kernel_spmd

    nc = _get_program()
    in_maps = _make_in_maps(hidden_states, seq_W, hid_W, cp_weight)
    res = run_bass_kernel_spmd(nc, in_maps, list(range(N_CORES)))

    out = np.empty((B, S, H), dtype=np.float32)
    for c in range(N_CORES):
        b, half = divmod(c, 2)
        out[b, half * SH:(half + 1) * SH, :] = (
            res.results[c]["out"].astype(np.float32))
    return out


# revision 18
# speedup vs baseline: 1.2339x; 1.0511x over previous
"""Trainium2 Bass kernel for nn_CPCircuitLayer.

Math: with all_indices the full cartesian grid (s = n // H, h = n % H),
    out[b, s, h] = sum_r seq_emb[b,s,r] * hid_emb[b,h,r] * cp[r]
                 = (seq_emb[b] @ diag(cp) @ hid_emb[b].T)[s, h]
where seq_emb[b] = X_b @ seq_W.T  (X_b = hidden_states[b], contract H)
      hid_emb[b] = X_b.T @ hid_W.T                        (contract S)

Sharding: 8 cores = (batch b, seq half) pairs. Each core receives X_b
fully (the hid factor contracts over all of S) with rows rotated so its
own seq half comes first, plus a host-transposed copy of that half
(xt = X_b[half].T), and computes
    hid_embT = (hid_W*cp) @ X_b          [R, H]
    seq_embT = seq_W @ X_b[half].T       [R, S/2]
    out_half = seq_embT.T @ hid_embT     [S/2, H]
writing its [512, 1024] slice of the output.

All device data is FP16 (PSUM accumulation FP32; output upcast on
host): rel err ~6e-4, half the HBM bytes, 3x the PE rate of fp32.
Schedule (both HWDGE queues share one SDMA pool at ~410 GB/s
aggregate, so per-queue chunks complete at ~205 GB/s in FIFO order):
xt streams FIRST so the seq factor completes mid-stream; x streams
per 256 KB k-tile so the final hid accumulation — the only work gated
by the whole input — starts as early as possible after the last tile.
The PE consumes tiles in expected arrival order (queues interleaved);
hid PSUM accumulation order is irrelevant. PE warmup matmuls on a
memset scratch begin right after the preamble (no data dependency) so
the HAM clock gate is at 2.4 GHz before real work. The tail fans
PSUM->SBUF fp16 casts across Vector, Scalar-ACT and GpSimd-Pool, and
the idle Sync engine dispatches all four output row DMAs. Every DMA
region is fully contiguous in DRAM (strided DRAM patterns halve HBM
throughput).
"""

import numpy as np

B, S, H, R = 4, 1024, 1024, 32
N_CORES = 8
SH = S // 2   # seq rows per core
KT = S // 128  # k-tiles over the contraction dims
MT = SH // 128  # row tiles in this core's seq half

_compiled = {}


def _np_fallback(hidden_states, all_indices, seq_W, hid_W, cp_weight):
    seq_emb = np.einsum("bsh,rh->bsr", hidden_states, seq_W)
    hid_emb = np.einsum("bsh,rs->bhr", hidden_states, hid_W)
    s_idx = all_indices[:, 0].astype(np.int64)
    h_idx = all_indices[:, 1].astype(np.int64)
    g_seq = seq_emb[:, s_idx, :]
    g_hid = hid_emb[:, h_idx, :]
    out = np.einsum("bnr,bnr,r->bn", g_seq, g_hid, cp_weight[0])
    return out.reshape(B, S, H).astype(np.float32)


def _chunk_pack(a, tiles_per_chunk):
    """[T*128, w] -> [T//c*128, c*w]: partition-major chunks of c k-tiles,
    each chunk a fully contiguous [128, c*w] DRAM block."""
    t = a.shape[0] // 128
    w = a.shape[1]
    c = tiles_per_chunk
    return np.ascontiguousarray(
        a.reshape(t // c, c, 128, w).transpose(0, 2, 1, 3)
        .reshape(t // c * 128, c * w))


def _wtile(w):
    """[K, R] -> [128, KT*R] tile layout, partition-contiguous."""
    return np.ascontiguousarray(
        w.reshape(KT, 128, R).transpose(1, 0, 2).reshape(128, KT * R))


def build_raw_program():
    import contextlib

    import concourse.bass as bass
    import concourse.mybir as mybir

    f16 = mybir.dt.float16
    f32 = mybir.dt.float32

    nc = bass.Bass("TRN2", target_bir_lowering=False, debug=False,
                   num_devices=N_CORES, enable_partition_id=False)

    # x: plain row-major k-tile stack (tile k = rows k*128..(k+1)*128)
    x_d = nc.dram_tensor("x", [KT * 128, H], f16, kind="ExternalInput")
    # xt: 2 host-packed chunks of 4 k-tiles, each contiguous [128, 4*SH]
    xt_d = nc.dram_tensor("xt", [2 * 128, 4 * SH], f16, kind="ExternalInput")
    w_d = nc.dram_tensor("w", [128, 2 * KT * R], f16, kind="ExternalInput")
    out_d = nc.dram_tensor("out", [SH, H], f16, kind="ExternalOutput")

    with contextlib.ExitStack() as _xs:
        E = _xs.enter_context
        w_t = E(nc.sbuf_tensor([128, 2 * KT * R], f16))  # [p, sw | hw]
        x_t = E(nc.sbuf_tensor([128, KT * H], f16))
        xt_t = E(nc.sbuf_tensor([128, KT * SH], f16))
        hid_sb = E(nc.sbuf_tensor([R, H], f16))
        seq_sb = E(nc.sbuf_tensor([R, SH], f16))
        o_sb = E(nc.sbuf_tensor([128, MT * H], f16))
        scr_sb = E(nc.sbuf_tensor([128, 512], f16))
        scr2_sb = E(nc.sbuf_tensor([128, R], f16))
        hid_ps = E(nc.psum_tensor([R, H], f32))        # 2 banks
        seq_ps = E(nc.psum_tensor([R, SH], f32))       # 1 bank
        o_ps = [E(nc.psum_tensor(f"o_ps{i}", [128, 512], f32))
                for i in range(5)]                     # 5 banks
        dma_sem = E(nc.semaphore("dma_sem"))
        w_sem = E(nc.semaphore("w_sem"))
        pe_sem = E(nc.semaphore("pe_sem"))
        dve_sem = E(nc.semaphore("dve_sem"))
        act_sem = E(nc.semaphore("act_sem"))
        gp_sem = E(nc.semaphore("gp_sem"))
        xs_sem = [E(nc.semaphore(f"xs_sem{j}")) for j in range(KT)]
        xtc_sem = [E(nc.semaphore(f"xtc_sem{j}")) for j in range(2)]
        block = E(nc.Block(no_gpsimd_drain=True))

        sw = lambda k: w_t.ap()[:, k * R:(k + 1) * R]
        hw = lambda k: w_t.ap()[:, KT * R + k * R:KT * R + (k + 1) * R]
        xk = lambda k, n: x_t.ap()[:, k * H + n * 512:k * H + (n + 1) * 512]
        xtk = lambda k: xt_t.ap()[:, k * SH:(k + 1) * SH]
        ob = lambda m, n: o_sb.ap()[:, m * H + n * 512:m * H + (n + 1) * 512]

        # queue plan (FIFO per queue, ~205 GB/s each while both active):
        #   sync:   w | xt k0-3 | x k0 k1 k2 k3   -> 1.66 MB
        #   scalar: xt k4-7 | x k4 k5 k6 k7       -> 1.53 MB
        # pe program order = expected arrival order:
        #   6 warmups; seq k0..7 (incs 1-8);
        #   hid k4,k0,k5,k1,k6,k2,k7,k3 (incs 9-24; n0 then n1 per k);
        #   finals j=0..7 (incs 25-32)
        # dve: 1 = seq_c, 2 = hid_c0, 3.. = out (m,0) copies
        # act: 1 = hid_c1, 2.. = out (m,1) copies
        # gp:  1 = scratch memset (GPSIMD cannot access PSUM)
        HID_ORDER = (4, 0, 5, 1, 6, 2, 7, 3)

        @block.sync
        def _(sync):
            sync.dma_start(out=w_t.ap(), in_=w_d[:]).then_inc(w_sem, 16)
            sync.dma_start(out=xt_t.ap()[:, 0:4 * SH],
                           in_=xt_d[0:128, :]).then_inc(xtc_sem[0], 16)
            for k in range(4):
                sync.dma_start(
                    out=x_t.ap()[:, k * H:(k + 1) * H],
                    in_=x_d[k * 128:(k + 1) * 128, :],
                ).then_inc(xs_sem[k], 16)
            # out rows, all on the (tail-idle) sync queue
            for m in range(MT):
                sync.wait_ge(dve_sem, 3 + m)    # (m,0) copy
                sync.wait_ge(act_sem, 2 + m)    # (m,1) copy
                sync.dma_start(
                    out=out_d[m * 128:(m + 1) * 128, :],
                    in_=o_sb.ap()[:, m * H:(m + 1) * H],
                ).then_inc(dma_sem, 16)
            sync.wait_ge(dma_sem, 16 * MT)

        @block.tensor
        def _(tensor):
            # HAM warmup on scratch, no input dependency: PE at 2.4 GHz
            # by the time real data lands
            tensor.wait_ge(gp_sem, 1)
            for _ in range(6):
                nc.tensor.matmul(o_ps[0].ap()[0:R, :], scr_sb.ap()[:, 0:R],
                                 scr_sb.ap(), start=True, stop=True)

            tensor.wait_ge(w_sem, 16)
            for k in range(KT):
                tensor.wait_ge(xtc_sem[k // 4], 16)
                nc.tensor.matmul(
                    seq_ps.ap(), sw(k), xtk(k),
                    start=(k == 0), stop=(k == KT - 1),
                ).then_inc(pe_sem, 1)

            for i, k in enumerate(HID_ORDER):
                tensor.wait_ge(xs_sem[k], 16)
                for n in range(2):
                    nc.tensor.matmul(
                        hid_ps.ap()[:, n * 512:(n + 1) * 512],
                        hw(k), xk(k, n),
                        start=(i == 0), stop=(i == len(HID_ORDER) - 1),
                    ).then_inc(pe_sem, 1)

            tensor.wait_ge(dve_sem, 2)   # seq_c + hid_c0
            for j in range(2 * MT):
                m, n = divmod(j, 2)
                if j == 1:
                    tensor.wait_ge(act_sem, 1)   # hid_c1 (odd j only)
                if j >= 5:
                    # WAR on recycled PSUM bank (5-deep rotation)
                    prev = j - 5
                    if prev % 2 == 0:
                        tensor.wait_ge(dve_sem, 3 + prev // 2)
                    else:
                        tensor.wait_ge(act_sem, 2 + (prev - 1) // 2)
                nc.tensor.matmul(
                    o_ps[j % 5].ap(),
                    seq_sb.ap()[:, m * 128:(m + 1) * 128],
                    hid_sb.ap()[:, n * 512:(n + 1) * 512],
                    start=True, stop=True,
                ).then_inc(pe_sem, 1)

        @block.vector
        def _(vector):
            vector.wait_ge(pe_sem, 8)
            nc.vector.tensor_copy(
                seq_sb.ap(), seq_ps.ap()).then_inc(dve_sem, 1)
            vector.wait_ge(pe_sem, 23)
            nc.vector.tensor_copy(
                hid_sb.ap()[:, 0:512],
                hid_ps.ap()[:, 0:512]).then_inc(dve_sem, 1)
            for j in range(0, 2 * MT, 2):    # out (m,0) copies
                m, n = divmod(j, 2)
                vector.wait_ge(pe_sem, 24 + j + 1)
                nc.vector.tensor_copy(
                    ob(m, n), o_ps[j % 5].ap()).then_inc(dve_sem, 1)

        @block.scalar
        def _(scalar):
            # second HWDGE queue: xt k4-7, then x k4..k7
            scalar.dma_start(out=xt_t.ap()[:, 4 * SH:8 * SH],
                            in_=xt_d[128:256, :]).then_inc(xtc_sem[1], 16)
            for k in range(4, 8):
                scalar.dma_start(
                    out=x_t.ap()[:, k * H:(k + 1) * H],
                    in_=x_d[k * 128:(k + 1) * 128, :],
                ).then_inc(xs_sem[k], 16)
            # dummy copy to pull the lazy ACT table load off the critical path
            scalar.wait_ge(w_sem, 16)
            nc.scalar.copy(scr2_sb.ap(), w_t.ap()[:, 0:R])
            scalar.wait_ge(pe_sem, 24)
            nc.scalar.copy(
                hid_sb.ap()[:, 512:1024],
                hid_ps.ap()[:, 512:1024]).then_inc(act_sem, 1)
            for j in range(1, 2 * MT, 2):    # out (m,1) copies
                m, n = divmod(j, 2)
                scalar.wait_ge(pe_sem, 24 + j + 1)
                nc.scalar.copy(
                    ob(m, n), o_ps[j % 5].ap()).then_inc(act_sem, 1)

        @block.gpsimd
        def _(gpsimd):
            gpsimd.memset(scr_sb.ap(), 0.0).then_inc(gp_sem, 1)

    return nc


def _get_program():
    if "nc" not in _compiled:
        _compiled["nc"] = build_raw_program()
    return _compiled["nc"]


def _make_in_maps(hidden_states, seq_W, hid_W, cp_weight):
    swT = _wtile(np.ascontiguousarray(seq_W.T))                    # [128, 256]
    hwT_rows = np.ascontiguousarray((hid_W * cp_weight[0][:, None]).T)  # [S, R]
    # per-half row rotation: own seq half first (hid contraction over S is
    # order-invariant as long as x rows and hw rows permute together)
    w_rot = [
        np.concatenate([swT, _wtile(np.concatenate(
            [hwT_rows[half * SH:], hwT_rows[:half * SH]], axis=0))],
            axis=1).astype(np.float16)
        for half in range(2)
    ]
    in_maps = []
    for c in range(N_CORES):
        b, half = divmod(c, 2)
        xb = hidden_states[b]
        if half:
            xb = np.concatenate([xb[SH:], xb[:SH]], axis=0)
        xb = xb.astype(np.float16)
        in_maps.append({
            "x": np.ascontiguousarray(xb),
            "xt": _chunk_pack(np.ascontiguousarray(xb[:SH, :].T), 4),
            "w": w_rot[half],
        })
    return in_maps


def kernel(hidden_states, all_indices, seq_W, hid_W, cp_weight):
    hidden_states = np.asarray(hidden_states, dtype=np.float32)
    seq_W = np.asarray(seq_W, dtype=np.float32)
    hid_W = np.asarray(hid_W, dtype=np.float32)
    cp_weight = np.asarray(cp_weight, dtype=np.float32)
    idx = np.asarray(all_indices)

    # The reference's all_indices is always the full cartesian grid; verify
    # cheaply and fall back to a host path if ever not.
    n = np.arange(S * H, dtype=idx.dtype)
    if idx.shape != (S * H, 2) or not (
        np.array_equal(idx[:, 0], n // H) and np.array_equal(idx[:, 1], n % H)
    ):
        return _np_fallback(hidden_states, idx, seq_W, hid_W, cp_weight)

    from concourse.bass_utils import run_bass_kernel_spmd

    nc = _get_program()
    in_maps = _make_in_maps(hidden_states, seq_W, hid_W, cp_weight)
    res = run_bass_kernel_spmd(nc, in_maps, list(range(N_CORES)))

    out = np.empty((B, S, H), dtype=np.float32)
    for c in range(N_CORES):
        b, half = divmod(c, 2)
        out[b, half * SH:(half + 1) * SH, :] = (
            res.results[c]["out"].astype(np.float32))
    return out


# revision 19
# speedup vs baseline: 1.2597x; 1.0209x over previous
"""Trainium2 Bass kernel for nn_CPCircuitLayer.

Math: with all_indices the full cartesian grid (s = n // H, h = n % H),
    out[b, s, h] = sum_r seq_emb[b,s,r] * hid_emb[b,h,r] * cp[r]
                 = (seq_emb[b] @ diag(cp) @ hid_emb[b].T)[s, h]
where seq_emb[b] = X_b @ seq_W.T  (X_b = hidden_states[b], contract H)
      hid_emb[b] = X_b.T @ hid_W.T                        (contract S)

Sharding: 8 cores = (batch b, seq half) pairs. Each core receives X_b
fully (the hid factor contracts over all of S) with rows rotated so its
own seq half comes first, plus a host-transposed copy of that half
(xt = X_b[half].T), and computes
    hid_embT = (hid_W*cp) @ X_b          [R, H]
    seq_embT = seq_W @ X_b[half].T       [R, S/2]
    out_half = seq_embT.T @ hid_embT     [S/2, H]
writing its [512, 1024] slice of the output.

All device data is FP16 (PSUM accumulation FP32; output upcast on
host): rel err ~6e-4, half the HBM bytes, 3x the PE rate of fp32.

Measured DMA behavior that shapes the schedule: the two HWDGE queues
share one SDMA pool (~410 GB/s aggregate, ~205 GB/s each while both
stream), each queue drains its dma_starts in FIFO order, and a
dma_start's completion semaphore lags its data by up to ~2 us under
HBM load — so FEW, LARGE transfers win, and the count per queue is
sized so transfer time, not completion handling, dominates. Each
queue carries 3 input DMAs; the tiny weight tensor rides the separate
GpSimd SWDGE queue. xt (seq factor) and x (hid factor) chunks are
interleaved across the queues so the PE always has work and the last
arrival gates only the short hid tail. Every DMA region is fully
contiguous in DRAM (the host packs per-chunk layouts; strided DRAM
patterns halve HBM throughput). PE warmup matmuls on a memset scratch
start right after the preamble so the HAM clock gate is at 2.4 GHz
before real work. The tail: hid PSUM->SBUF casts split across Vector
and Scalar-ACT, final matmuls through a 5-deep PSUM bank rotation,
out-copies alternating DVE/ACT, and the tail-idle Sync engine
dispatches all four output row DMAs.
"""

import numpy as np

B, S, H, R = 4, 1024, 1024, 32
N_CORES = 8
SH = S // 2   # seq rows per core
KT = S // 128  # k-tiles over the contraction dims
MT = SH // 128  # row tiles in this core's seq half

_compiled = {}


def _np_fallback(hidden_states, all_indices, seq_W, hid_W, cp_weight):
    seq_emb = np.einsum("bsh,rh->bsr", hidden_states, seq_W)
    hid_emb = np.einsum("bsh,rs->bhr", hidden_states, hid_W)
    s_idx = all_indices[:, 0].astype(np.int64)
    h_idx = all_indices[:, 1].astype(np.int64)
    g_seq = seq_emb[:, s_idx, :]
    g_hid = hid_emb[:, h_idx, :]
    out = np.einsum("bnr,bnr,r->bn", g_seq, g_hid, cp_weight[0])
    return out.reshape(B, S, H).astype(np.float32)


def _pm(a):
    """[t*128, w] -> [128, t*w] partition-major pack (one contiguous
    chunk: partition p holds row p of every k-tile, tiles adjacent)."""
    t = a.shape[0] // 128
    return np.ascontiguousarray(
        a.reshape(t, 128, a.shape[1]).transpose(1, 0, 2).reshape(128, -1))


def _wtile(w):
    """[K, R] -> [128, KT*R] tile layout, partition-contiguous."""
    return np.ascontiguousarray(
        w.reshape(KT, 128, R).transpose(1, 0, 2).reshape(128, KT * R))


def build_raw_program():
    import contextlib

    import concourse.bass as bass
    import concourse.mybir as mybir

    f16 = mybir.dt.float16
    f32 = mybir.dt.float32

    nc = bass.Bass("TRN2", target_bir_lowering=False, debug=False,
                   num_devices=N_CORES, enable_partition_id=False)

    # per-chunk contiguous partition-major packs (host-side _pm)
    x_a = nc.dram_tensor("x_a", [128, 3 * H], f16, kind="ExternalInput")
    x_b = nc.dram_tensor("x_b", [128, 1 * H], f16, kind="ExternalInput")
    x_c = nc.dram_tensor("x_c", [128, 3 * H], f16, kind="ExternalInput")
    x_e = nc.dram_tensor("x_e", [128, 1 * H], f16, kind="ExternalInput")
    xt_a = nc.dram_tensor("xt_a", [128, 4 * SH], f16, kind="ExternalInput")
    xt_b = nc.dram_tensor("xt_b", [128, 4 * SH], f16, kind="ExternalInput")
    w_d = nc.dram_tensor("w", [128, 2 * KT * R], f16, kind="ExternalInput")
    out_d = nc.dram_tensor("out", [SH, H], f16, kind="ExternalOutput")

    with contextlib.ExitStack() as _xs:
        E = _xs.enter_context
        w_t = E(nc.sbuf_tensor([128, 2 * KT * R], f16))  # [p, sw | hw]
        x_t = E(nc.sbuf_tensor([128, KT * H], f16))      # tiles k0..k7
        xt_t = E(nc.sbuf_tensor([128, KT * SH], f16))
        hid_sb = E(nc.sbuf_tensor([R, H], f16))
        seq_sb = E(nc.sbuf_tensor([R, SH], f16))
        o_sb = E(nc.sbuf_tensor([128, MT * H], f16))
        scr_sb = E(nc.sbuf_tensor([128, 512], f16))
        scr2_sb = E(nc.sbuf_tensor([128, R], f16))
        hid_ps = E(nc.psum_tensor([R, H], f32))        # 2 banks
        seq_ps = E(nc.psum_tensor([R, SH], f32))       # 1 bank
        o_ps = [E(nc.psum_tensor(f"o_ps{i}", [128, 512], f32))
                for i in range(5)]                     # 5 banks
        dma_sem = E(nc.semaphore("dma_sem"))
        w_sem = E(nc.semaphore("w_sem"))
        pe_sem = E(nc.semaphore("pe_sem"))
        dve_sem = E(nc.semaphore("dve_sem"))
        act_sem = E(nc.semaphore("act_sem"))
        gp_sem = E(nc.semaphore("gp_sem"))
        xa_sem = E(nc.semaphore("xa_sem"))
        xb_sem = E(nc.semaphore("xb_sem"))
        xc_sem = E(nc.semaphore("xc_sem"))
        xe_sem = E(nc.semaphore("xe_sem"))
        xta_sem = E(nc.semaphore("xta_sem"))
        xtb_sem = E(nc.semaphore("xtb_sem"))
        block = E(nc.Block(no_gpsimd_drain=True))

        sw = lambda k: w_t.ap()[:, k * R:(k + 1) * R]
        hw = lambda k: w_t.ap()[:, KT * R + k * R:KT * R + (k + 1) * R]
        xk = lambda k, n: x_t.ap()[:, k * H + n * 512:k * H + (n + 1) * 512]
        xtk = lambda k: xt_t.ap()[:, k * SH:(k + 1) * SH]
        ob = lambda m, n: o_sb.ap()[:, m * H + n * 512:m * H + (n + 1) * 512]

        # queue plan (FIFO per queue; ~205 GB/s each while both stream):
        #   sync:   xt_a(k0-3) | x_a(k0-2) | x_b(k3)   -> 1.5 MB
        #   scalar: x_c(k4-6) | xt_b(k4-7) | x_e(k7)   -> 1.5 MB
        #   gpsimd: w (SWDGE)
        # pe program order = expected arrival order:
        #   6 warmups; seq k0-3 (incs 1-4); hid k4,5,6 (5-10);
        #   seq k4-7 (11-14); hid k0,1,2 (15-20); hid k3 (21-22);
        #   hid k7 (23-24); finals j=0..7 (25-32)
        # dve: 1 = seq_c, 2 = hid_c0, 3.. = out (m,0) copies
        # act: 1 = hid_c1, 2.. = out (m,1) copies

        @block.sync
        def _(sync):
            sync.dma_start(out=xt_t.ap()[:, 0:4 * SH],
                           in_=xt_a[:]).then_inc(xta_sem, 16)
            sync.dma_start(out=x_t.ap()[:, 0:3 * H],
                           in_=x_a[:]).then_inc(xa_sem, 16)
            sync.dma_start(out=x_t.ap()[:, 3 * H:4 * H],
                           in_=x_b[:]).then_inc(xb_sem, 16)
            for m in range(MT):
                sync.wait_ge(dve_sem, 3 + m)    # (m,0) copy
                sync.wait_ge(act_sem, 2 + m)    # (m,1) copy
                sync.dma_start(
                    out=out_d[m * 128:(m + 1) * 128, :],
                    in_=o_sb.ap()[:, m * H:(m + 1) * H],
                ).then_inc(dma_sem, 16)
            sync.wait_ge(dma_sem, 16 * MT)

        @block.tensor
        def _(tensor):
            # HAM warmup on scratch, no input dependency: PE at 2.4 GHz
            # by the time real data lands
            tensor.wait_ge(gp_sem, 1)
            for _ in range(6):
                nc.tensor.matmul(o_ps[0].ap()[0:R, :], scr_sb.ap()[:, 0:R],
                                 scr_sb.ap(), start=True, stop=True)

            tensor.wait_ge(w_sem, 16)
            tensor.wait_ge(xta_sem, 16)
            for k in range(4):
                nc.tensor.matmul(
                    seq_ps.ap(), sw(k), xtk(k),
                    start=(k == 0), stop=False,
                ).then_inc(pe_sem, 1)

            def hid(k, first, last):
                for n in range(2):
                    nc.tensor.matmul(
                        hid_ps.ap()[:, n * 512:(n + 1) * 512],
                        hw(k), xk(k, n),
                        start=first, stop=last,
                    ).then_inc(pe_sem, 1)

            tensor.wait_ge(xc_sem, 16)
            for k in (4, 5, 6):
                hid(k, k == 4, False)

            tensor.wait_ge(xtb_sem, 16)
            for k in range(4, 8):
                nc.tensor.matmul(
                    seq_ps.ap(), sw(k), xtk(k),
                    start=False, stop=(k == 7),
                ).then_inc(pe_sem, 1)

            tensor.wait_ge(xa_sem, 16)
            for k in (0, 1, 2):
                hid(k, False, False)
            tensor.wait_ge(xb_sem, 16)
            hid(3, False, False)
            tensor.wait_ge(xe_sem, 16)
            hid(7, False, True)

            tensor.wait_ge(dve_sem, 2)   # seq_c + hid_c0
            for j in range(2 * MT):
                m, n = divmod(j, 2)
                if j == 1:
                    tensor.wait_ge(act_sem, 1)   # hid_c1 (odd j only)
                if j >= 5:
                    # WAR on recycled PSUM bank (5-deep rotation)
                    prev = j - 5
                    if prev % 2 == 0:
                        tensor.wait_ge(dve_sem, 3 + prev // 2)
                    else:
                        tensor.wait_ge(act_sem, 2 + (prev - 1) // 2)
                nc.tensor.matmul(
                    o_ps[j % 5].ap(),
                    seq_sb.ap()[:, m * 128:(m + 1) * 128],
                    hid_sb.ap()[:, n * 512:(n + 1) * 512],
                    start=True, stop=True,
                ).then_inc(pe_sem, 1)

        @block.vector
        def _(vector):
            vector.wait_ge(pe_sem, 14)   # seq k7 done
            nc.vector.tensor_copy(
                seq_sb.ap(), seq_ps.ap()).then_inc(dve_sem, 1)
            vector.wait_ge(pe_sem, 23)   # hid k7 n0 done
            nc.vector.tensor_copy(
                hid_sb.ap()[:, 0:512],
                hid_ps.ap()[:, 0:512]).then_inc(dve_sem, 1)
            for j in range(0, 2 * MT, 2):    # out (m,0) copies
                m, n = divmod(j, 2)
                vector.wait_ge(pe_sem, 24 + j + 1)
                nc.vector.tensor_copy(
                    ob(m, n), o_ps[j % 5].ap()).then_inc(dve_sem, 1)

        @block.scalar
        def _(scalar):
            # second HWDGE queue
            scalar.dma_start(out=x_t.ap()[:, 4 * H:7 * H],
                            in_=x_c[:]).then_inc(xc_sem, 16)
            scalar.dma_start(out=xt_t.ap()[:, 4 * SH:8 * SH],
                            in_=xt_b[:]).then_inc(xtb_sem, 16)
            scalar.dma_start(out=x_t.ap()[:, 7 * H:8 * H],
                            in_=x_e[:]).then_inc(xe_sem, 16)
            # dummy copy to pull the lazy ACT table load off the critical path
            scalar.wait_ge(w_sem, 16)
            nc.scalar.copy(scr2_sb.ap(), w_t.ap()[:, 0:R])
            scalar.wait_ge(pe_sem, 24)   # hid k7 n1 done
            nc.scalar.copy(
                hid_sb.ap()[:, 512:1024],
                hid_ps.ap()[:, 512:1024]).then_inc(act_sem, 1)
            for j in range(1, 2 * MT, 2):    # out (m,1) copies
                m, n = divmod(j, 2)
                scalar.wait_ge(pe_sem, 24 + j + 1)
                nc.scalar.copy(
                    ob(m, n), o_ps[j % 5].ap()).then_inc(act_sem, 1)

        @block.gpsimd
        def _(gpsimd):
            gpsimd.memset(scr_sb.ap(), 0.0).then_inc(gp_sem, 1)
            gpsimd.dma_start(out=w_t.ap(), in_=w_d[:]).then_inc(w_sem, 16)

    return nc


def _get_program():
    if "nc" not in _compiled:
        _compiled["nc"] = build_raw_program()
    return _compiled["nc"]


def _make_in_maps(hidden_states, seq_W, hid_W, cp_weight):
    swT = _wtile(np.ascontiguousarray(seq_W.T))                    # [128, 256]
    hwT_rows = np.ascontiguousarray((hid_W * cp_weight[0][:, None]).T)  # [S, R]
    # per-half row rotation: own seq half first (hid contraction over S is
    # order-invariant as long as x rows and hw rows permute together)
    w_rot = [
        np.concatenate([swT, _wtile(np.concatenate(
            [hwT_rows[half * SH:], hwT_rows[:half * SH]], axis=0))],
            axis=1).astype(np.float16)
        for half in range(2)
    ]
    in_maps = []
    for c in range(N_CORES):
        b, half = divmod(c, 2)
        xb = hidden_states[b]
        if half:
            xb = np.concatenate([xb[SH:], xb[:SH]], axis=0)
        xb = xb.astype(np.float16)
        xt = np.ascontiguousarray(xb[:SH, :].T)   # [H, SH]
        in_maps.append({
            "x_a": _pm(xb[0:384]),
            "x_b": _pm(xb[384:512]),
            "x_c": _pm(xb[512:896]),
            "x_e": _pm(xb[896:1024]),
            "xt_a": _pm(xt[0:512]),
            "xt_b": _pm(xt[512:1024]),
            "w": w_rot[half],
        })
    return in_maps


def kernel(hidden_states, all_indices, seq_W, hid_W, cp_weight):
    hidden_states = np.asarray(hidden_states, dtype=np.float32)
    seq_W = np.asarray(seq_W, dtype=np.float32)
    hid_W = np.asarray(hid_W, dtype=np.float32)
    cp_weight = np.asarray(cp_weight, dtype=np.float32)
    idx = np.asarray(all_indices)

    # The reference's all_indices is always the full cartesian grid; verify
    # cheaply and fall back to a host path if ever not.
    n = np.arange(S * H, dtype=idx.dtype)
    if idx.shape != (S * H, 2) or not (
        np.array_equal(idx[:, 0], n // H) and np.array_equal(idx[:, 1], n % H)
    ):
        return _np_fallback(hidden_states, idx, seq_W, hid_W, cp_weight)

    from concourse.bass_utils import run_bass_kernel_spmd

    nc = _get_program()
    in_maps = _make_in_maps(hidden_states, seq_W, hid_W, cp_weight)
    res = run_bass_kernel_spmd(nc, in_maps, list(range(N_CORES)))

    out = np.empty((B, S, H), dtype=np.float32)
    for c in range(N_CORES):
        b, half = divmod(c, 2)
        out[b, half * SH:(half + 1) * SH, :] = (
            res.results[c]["out"].astype(np.float32))
    return out
